# revision 20
# baseline (speedup 1.0000x reference)
"""Causal single-head attention (B=4, S=4096, D=1024) on 8 trn2 NeuronCores.

Sharding: 2 cores per batch element. Each core owns 16 interleaved 128-row
query blocks (core parity k takes global blocks g = 2t + k, t = 0..15), which
balances the causal triangle exactly: local block t attends to key columns
[0, (2t+2)*128), identical extent on every core, so one SPMD program serves
all 8 cores. The causal boundary only affects the last 256 key columns of
each block's extent; a per-core constant additive mask [128, 256] handles it.

Per core, on device (default variant v12):
  Projections are bf16 matmuls with fp32 PSUM accumulation, V interleaved
  with the attention phase so its slab DMAs prefetch behind score matmuls.
  q/k are stored as e4m3 fp8 (scaled by 8) in a paired layout and the score
  matmuls run fp8 DoubleRow (2 o-chunks of the D=1024 contraction per
  instruction); the combined scale is folded into the ACT exp (scale
  1/(32*64)) which also accumulates the row sums. PE-transpose of attn
  blocks (lagging one score group), attn^T-stationary bf16 PV matmuls, and a
  per-row 1/sum rescale fused into the PSUM->SBUF eviction.

Variant v13 additionally projects K/V only for each pair core's own half of
the sequence and exchanges halves via a 2-core AllGather through DRAM
(correct on HW, same numerics), but on this runtime the collectives cost
more than the redundant projection work they remove and desync the PJRT
mesh in repeated-body NEFFs, so v12 stays the default.
"""

import math

import numpy as np
import ml_dtypes

from concourse import bacc, mybir, tile
from concourse.bass_utils import run_bass_kernel_spmd

B, S, D = 4, 4096, 1024
NCORES = 8
P = 128
DK = D // P          # 8 contraction chunks of 128 over d_in / d_out
NQB = (S // 2) // P  # 16 local query blocks per core
NEG = -1.0e30

_CACHE = {}
# v12 = fp8 DoubleRow scores, everything else bf16, no collectives.
# v13 (K/V pair-exchange via AllGather) is correct on HW but the collectives
# through this runtime's PJRT path cost more than the projection work they
# save and desync the mesh in repeated-body NEFFs; not worth it here.
VARIANT = "v12"


def _build_program(reps=1, phase="full", variant="v0"):
    bf16 = mybir.dt.bfloat16
    f32 = mybir.dt.float32
    fp8 = mybir.dt.float8e4
    fp8sc = variant in ("v12", "v13", "v14")
    kvex = variant in ("v13",)  # pair-exchange K/V halves via AllGather
    SQ = 8.0  # q/k fp8 quantization scale; folded out of exp via 1/(32*SQ^2)
    nc = bacc.Bacc(
        "TRN2",
        target_bir_lowering=False,
        debug=False,
        num_devices=NCORES,
    )

    if kvex:
        # per-core half of x^T (cols parity*S/2 ..): K/V are projected for
        # these rows only and the halves exchanged pairwise via AllGather.
        xT_d = nc.dram_tensor("xTh", [DK, P, S // 2], bf16, kind="ExternalInput")
    else:
        xT_d = nc.dram_tensor("xT", [DK, P, S], bf16, kind="ExternalInput")
    xTq_d = nc.dram_tensor("xTq", [DK, P, S // 2], bf16, kind="ExternalInput")
    wqT_d = nc.dram_tensor("wqT", [DK, P, D], bf16, kind="ExternalInput")
    wkT_d = nc.dram_tensor("wkT", [DK, P, D], bf16, kind="ExternalInput")
    wvT_d = nc.dram_tensor("wvT", [DK, P, D], bf16, kind="ExternalInput")
    mask_d = nc.dram_tensor("mask", [P, 2 * P], bf16, kind="ExternalInput")
    ident_d = nc.dram_tensor("ident", [P, P], bf16, kind="ExternalInput")
    out_d = nc.dram_tensor("out", [NQB, P, D], f32, kind="ExternalOutput")

    with tile.TileContext(nc) as tc:
        with (
            tc.tile_pool(name="const", bufs=1) as constp,
            tc.tile_pool(name="w", bufs=(11 if variant == "v4" else (8 if variant in ("v7", "v10") else 9))) as wp,
            tc.tile_pool(name="slab", bufs=(10 if variant == "v4" else (17 if variant == "v10" else (15 if variant in ("v9", "v12", "v14") else (14 if variant in ("v8", "v11") else 12))))) as slabp,
            tc.tile_pool(name="qT", bufs=1) as qTp,
            tc.tile_pool(name="kT", bufs=1) as kTp,
            tc.tile_pool(name="v", bufs=1) as vp,
            tc.tile_pool(name="attn", bufs=(2 if variant in ("v4", "v7", "v8", "v9", "v10", "v11", "v12", "v14") else 3)) as attnp,
            tc.tile_pool(name="attnT", bufs=1) as attnTp,
            tc.tile_pool(name="stat", bufs=(1 if variant in ("v9", "v10", "v11", "v12", "v14") else 2)) as statp,
            tc.tile_pool(name="outst", bufs=(2 if variant == "v7" else 1)) as outp,
            tc.tile_pool(name="kH", bufs=1) as kHp,
            tc.tile_pool(name="bounce", bufs=3) as bouncep,
            tc.tile_pool(name="dram", bufs=1, space="DRAM") as dramp,
            tc.tile_pool(name="psmm", bufs=(4 if variant in ("v1", "v6", "v14") else 3),
                         space="PSUM") as psmm,
            tc.tile_pool(name="pstr", bufs=(3 if variant in ("v2", "v5", "v7", "v8", "v9", "v10", "v11", "v12") else 2),
                         space="PSUM") as pstr,
            tc.tile_pool(name="pspv", bufs=(2 if variant in ("v1", "v2", "v5", "v6", "v7", "v8", "v9", "v10", "v11", "v12", "v14") else 3),
                         space="PSUM") as pspv,
        ):
            mask_t = constp.tile([P, 2 * P], bf16, tag="mask", name="mask_t")
            nc.sync.dma_start(mask_t[:], mask_d[:])
            ident_t = constp.tile([P, P], bf16, tag="ident", name="ident_t")
            nc.sync.dma_start(ident_t[:], ident_d[:])

            def load_w(wdram):
                ws = []
                for d in range(DK):
                    w = wp.tile([P, D], bf16, tag="w", name=f"w{d}")
                    nc.sync.dma_start(w[:], wdram[d])
                    ws.append(w)
                return ws

            def load_slab(src, c0, cw):
                slab = []
                for d in range(DK):
                    t = slabp.tile([P, 512], bf16, tag="slab", name=f"slab{d}")
                    nc.sync.dma_start(t[:, :cw], src[d][:, c0 : c0 + cw])
                    slab.append(t)
                return slab

            for rep in range(reps):
                if kvex and phase == "full":
                    # ---- v13: each pair core projects K/V only for its own
                    # half of the sequence (rank r of the replica group owns
                    # s in [r*S/2, (r+1)*S/2)); halves exchanged by AllGather.
                    # Order: V first, K+exchange, then Q, so both collectives
                    # complete under projection compute before scores/PV.
                    HC = S // 2
                    rgs = [[2 * b, 2 * b + 1] for b in range(NCORES // 2)]

                    # V projection (own half) staged straight to DRAM
                    wv = load_w(wvT_d)
                    vex_in = dramp.tile([16 * P, D], bf16, tag="vex_in", name="vex_in")
                    for sg in range(4):
                        slab = load_slab(xT_d, sg * 512, 512)
                        for sh in range(8):
                            ss, h = sh // 2, sh % 2
                            ps = psmm.tile([P, 512], f32, tag="psmm", name="ps")
                            for d in range(DK):
                                nc.tensor.matmul(
                                    ps[:],
                                    slab[d][:, ss * P : (ss + 1) * P],
                                    wv[d][:, h * 512 : (h + 1) * 512],
                                    start=(d == 0),
                                    stop=(d == DK - 1),
                                )
                            bnc = bouncep.tile([P, 512], bf16, tag="bounce", name="bnc")
                            nc.vector.tensor_copy(bnc[:], ps[:])
                            nc.sync.dma_start(
                                vex_in[
                                    (sg * 4 + ss) * P : (sg * 4 + ss + 1) * P,
                                    h * 512 : (h + 1) * 512,
                                ],
                                bnc[:],
                            )
                    vex_out = dramp.tile([32 * P, D], bf16, tag="vex_out", name="vex_out")
                    nc.gpsimd.collective_compute(
                        "AllGather",
                        mybir.AluOpType.bypass,
                        replica_groups=rgs,
                        ins=[vex_in.opt()],
                        outs=[vex_out.opt()],
                    )

                    # K projection (own half) -> fp8 pair staging -> exchange
                    wk = load_w(wkT_d)
                    kH = [kHp.tile([P, S], fp8, tag=f"kH{j}", name=f"kH{j}")
                          for j in range(DK // 2)]
                    for sg in range(4):
                        slab = load_slab(xT_d, sg * 512, 512)
                        for o in range(DK):
                            ps = psmm.tile([P, 512], f32, tag="psmm", name="ps")
                            for d in range(DK):
                                nc.tensor.matmul(
                                    ps[:],
                                    wk[d][:, o * P : (o + 1) * P],
                                    slab[d][:],
                                    start=(d == 0),
                                    stop=(d == DK - 1),
                                )
                            base = (o % 2) * HC + sg * 512
                            nc.scalar.activation(
                                kH[o // 2][:, base : base + 512],
                                ps[:],
                                mybir.ActivationFunctionType.Copy,
                                scale=SQ,
                            )
                    kex_in = dramp.tile([4 * P, S], fp8, tag="kex_in", name="kex_in")
                    for j in range(DK // 2):
                        nc.sync.dma_start(kex_in[j * P : (j + 1) * P, :], kH[j][:])
                    kex_out = dramp.tile([8 * P, S], fp8, tag="kex_out", name="kex_out")
                    nc.gpsimd.collective_compute(
                        "AllGather",
                        mybir.AluOpType.bypass,
                        replica_groups=rgs,
                        ins=[kex_in.opt()],
                        outs=[kex_out.opt()],
                    )

                    # Q projection -> fp8 pair layout (as v12)
                    wq = load_w(wqT_d)
                    qT8 = [qTp.tile([P, S], fp8, tag=f"qT8{j}", name=f"qT8{j}")
                           for j in range(DK // 2)]
                    for mg in range(4):
                        slab = load_slab(xTq_d, mg * 512, 512)
                        for o in range(DK):
                            ps = psmm.tile([P, 512], f32, tag="psmm", name="ps")
                            for d in range(DK):
                                nc.tensor.matmul(
                                    ps[:],
                                    wq[d][:, o * P : (o + 1) * P],
                                    slab[d][:],
                                    start=(d == 0),
                                    stop=(d == DK - 1),
                                )
                            base = (o % 2) * HC + mg * 512
                            nc.scalar.activation(
                                qT8[o // 2][:, base : base + 512],
                                ps[:],
                                mybir.ActivationFunctionType.Copy,
                                scale=SQ,
                            )

                    # gather readbacks: kT8 pair layout over full S; v blocks
                    kT8 = [kTp.tile([P, 2 * S], fp8, tag=f"kT8{j}", name=f"kT8{j}")
                           for j in range(DK // 2)]
                    for j in range(DK // 2):
                        for r in range(2):
                            for m in range(2):
                                nc.sync.dma_start(
                                    kT8[j][:, m * S + r * HC : m * S + r * HC + HC],
                                    kex_out[
                                        (r * 4 + j) * P : (r * 4 + j + 1) * P,
                                        m * HC : (m + 1) * HC,
                                    ],
                                )
                    v = [vp.tile([P, D], bf16, tag=f"v{j}", name=f"v{j}")
                         for j in range(S // P)]
                    for g in range(S // P):
                        nc.sync.dma_start(v[g][:], vex_out[g * P : (g + 1) * P, :])

                    # attention: identical numerics to v12, pipelined by one
                    # query block with transposes lagging one score group.
                    pends = []

                    def _flush(all=True):
                        while pends and (all or len(pends) > 1):
                            attn_g, attnT_t, g0, gw0 = pends.pop(0)
                            for jj in range(gw0 // P):
                                j = g0 * 4 + jj
                                pst = pstr.tile([P, P], bf16, tag="pstr", name="pst")
                                nc.tensor.transpose(
                                    pst[:], attn_g[:, jj * P : (jj + 1) * P], ident_t[:]
                                )
                                nc.vector.tensor_copy(
                                    attnT_t[:, j * P : (j + 1) * P], pst[:]
                                )

                    blocks = {}

                    def emit_scores(t):
                        nsb = 2 * t + 2
                        ext = nsb * P
                        G = math.ceil(ext / 512)
                        sums = statp.tile([P, G], f32, tag="sums", name="sums")
                        attnT = attnTp.tile([P, ext], bf16, tag="attnT", name="attnT")
                        for g in range(G):
                            gw = min(512, ext - g * 512)
                            ps = psmm.tile([P, gw], f32, tag="psmm", name="ps")
                            for jo in range(DK // 2):
                                lhs = qT8[jo][:].rearrange(
                                    "p (two m) -> p two m", two=2
                                )[:, :, t * P : (t + 1) * P]
                                rhs = kT8[jo][:].rearrange(
                                    "p (two n) -> p two n", two=2
                                )[:, :, g * 512 : g * 512 + gw]
                                nc.tensor.matmul(
                                    ps[:],
                                    lhs,
                                    rhs,
                                    start=(jo == 0),
                                    stop=(jo == DK // 2 - 1),
                                    perf_mode=mybir.MatmulPerfMode.DoubleRow,
                                )
                            if g == G - 1:
                                nc.vector.tensor_add(
                                    ps[:, gw - 2 * P : gw],
                                    ps[:, gw - 2 * P : gw],
                                    mask_t[:],
                                )
                            attn_g = attnp.tile([P, gw], bf16, tag="attn", name="attn_g")
                            nc.scalar.activation(
                                attn_g[:],
                                ps[:],
                                mybir.ActivationFunctionType.Exp,
                                scale=1.0 / (32.0 * SQ * SQ),
                                accum_out=sums[:, g : g + 1],
                            )
                            pends.append((attn_g, attnT, g, gw))
                            _flush(all=False)
                        tot = statp.tile([P, 1], f32, tag="tot", name="tot")
                        nc.vector.reduce_sum(tot[:], sums[:], axis=mybir.AxisListType.X)
                        recip = statp.tile([P, 1], f32, tag="recip", name="recip")
                        nc.vector.reciprocal(recip[:], tot[:])
                        blocks[t] = (attnT, recip, nsb)

                    def emit_pv(t):
                        attnT_p, recip_p, nsb_p = blocks.pop(t)
                        outst = outp.tile([P, D], f32, tag="outst", name="outst")
                        for h in range(2):
                            ps = pspv.tile([P, 512], f32, tag="pspv", name="pspv")
                            for j in range(nsb_p):
                                nc.tensor.matmul(
                                    ps[:],
                                    attnT_p[:, j * P : (j + 1) * P],
                                    v[j][:, h * 512 : (h + 1) * 512],
                                    start=(j == 0),
                                    stop=(j == nsb_p - 1),
                                )
                            nc.vector.tensor_scalar_mul(
                                outst[:, h * 512 : (h + 1) * 512], ps[:], recip_p[:]
                            )
                        nc.sync.dma_start(out_d[t], outst[:])

                    emit_scores(0)
                    for t in range(1, NQB):
                        emit_scores(t)
                        emit_pv(t - 1)
                    _flush()
                    emit_pv(NQB - 1)
                    continue

                # ---- Q projection: qT[o, m] for 2048 local query rows
                wq = load_w(wqT_d)
                if fp8sc:
                    # fp8 pair layout for DoubleRow: tile j holds o-chunk 2j in
                    # free [0, S/2) and o-chunk 2j+1 in [S/2, S), values scaled
                    # by SQ before the e4m3 cast.
                    qT8 = [qTp.tile([P, S], fp8, tag=f"qT8{j}", name=f"qT8{j}")
                           for j in range(DK // 2)]
                else:
                    qT = [qTp.tile([P, S // 2], bf16, tag=f"qT{o}", name=f"qT{o}") for o in range(DK)]
                for mg in range(4):
                    slab = load_slab(xTq_d, mg * 512, 512)
                    for o in range(DK):
                        ps = psmm.tile([P, 512], f32, tag="psmm", name="ps")
                        for d in range(DK):
                            nc.tensor.matmul(
                                ps[:],
                                wq[d][:, o * P : (o + 1) * P],
                                slab[d][:],
                                start=(d == 0),
                                stop=(d == DK - 1),
                            )
                        if fp8sc:
                            base = (o % 2) * (S // 2) + mg * 512
                            nc.scalar.activation(
                                qT8[o // 2][:, base : base + 512],
                                ps[:],
                                mybir.ActivationFunctionType.Copy,
                                scale=SQ,
                            )
                        elif variant == "v1" and o % 2 == 1:
                            nc.scalar.copy(qT[o][:, mg * 512 : (mg + 1) * 512], ps[:])
                        else:
                            nc.vector.tensor_copy(qT[o][:, mg * 512 : (mg + 1) * 512], ps[:])

                # ---- K projection: kT[o, s] (full rows, or own half under kvex)
                wk = load_w(wkT_d)
                if kvex:
                    # pair-layout fp8 staging for this core's half of K
                    kH = [kHp.tile([P, S], fp8, tag=f"kH{j}", name=f"kH{j}")
                          for j in range(DK // 2)]
                elif fp8sc:
                    kT8 = [kTp.tile([P, 2 * S], fp8, tag=f"kT8{j}", name=f"kT8{j}")
                           for j in range(DK // 2)]
                else:
                    kT = [kTp.tile([P, S], bf16, tag=f"kT{o}", name=f"kT{o}") for o in range(DK)]
                for sg in range(4 if kvex else 8):
                    slab = load_slab(xT_d, sg * 512, 512)
                    for o in range(DK):
                        ps = psmm.tile([P, 512], f32, tag="psmm", name="ps")
                        for d in range(DK):
                            nc.tensor.matmul(
                                ps[:],
                                wk[d][:, o * P : (o + 1) * P],
                                slab[d][:],
                                start=(d == 0),
                                stop=(d == DK - 1),
                            )
                        if kvex:
                            base = (o % 2) * (S // 2) + sg * 512
                            nc.scalar.activation(
                                kH[o // 2][:, base : base + 512],
                                ps[:],
                                mybir.ActivationFunctionType.Copy,
                                scale=SQ,
                            )
                        elif fp8sc:
                            base = (o % 2) * S + sg * 512
                            nc.scalar.activation(
                                kT8[o // 2][:, base : base + 512],
                                ps[:],
                                mybir.ActivationFunctionType.Copy,
                                scale=SQ,
                            )
                        elif variant == "v1" and o % 2 == 1:
                            nc.scalar.copy(kT[o][:, sg * 512 : (sg + 1) * 512], ps[:])
                        else:
                            nc.vector.tensor_copy(kT[o][:, sg * 512 : (sg + 1) * 512], ps[:])

                if kvex:
                    # ---- exchange K halves within the batch pair (rank r of
                    # the replica group owns s in [r*S/2, (r+1)*S/2))
                    HC = S // 2
                    rgs = [[2 * b, 2 * b + 1] for b in range(NCORES // 2)]
                    kex_in = dramp.tile([4 * P, S], fp8, tag="kex_in", name="kex_in")
                    for j in range(DK // 2):
                        nc.sync.dma_start(kex_in[j * P : (j + 1) * P, :], kH[j][:])
                    kex_out = dramp.tile([8 * P, S], fp8, tag="kex_out", name="kex_out")
                    nc.gpsimd.collective_compute(
                        "AllGather",
                        mybir.AluOpType.bypass,
                        replica_groups=rgs,
                        ins=[kex_in.opt()],
                        outs=[kex_out.opt()],
                    )
                    kT8 = [kTp.tile([P, 2 * S], fp8, tag=f"kT8{j}", name=f"kT8{j}")
                           for j in range(DK // 2)]
                    for j in range(DK // 2):
                        for r in range(2):
                            for m in range(2):
                                nc.sync.dma_start(
                                    kT8[j][:, m * S + r * HC : m * S + r * HC + HC],
                                    kex_out[
                                        (r * 4 + j) * P : (r * 4 + j + 1) * P,
                                        m * HC : (m + 1) * HC,
                                    ],
                                )

                if variant in ("v3", "v4", "v5", "v6", "v7", "v8", "v9", "v10", "v11", "v12", "v14") and phase == "full":
                    # ---- V projection interleaved with attention: scores
                    # only need qT/kT, so V slab DMAs prefetch behind ~230us
                    # of score matmuls instead of stalling the PE in a
                    # dedicated V pass.
                    wv = load_w(wvT_d)
                    v = [
                        vp.tile([P, D], bf16, tag=f"v{j}", name=f"v{j}")
                        for j in range(S // P)
                    ]
                    blocks = {}
                    pends = []
                    pend_depth = 2 if variant == "v11" else 1

                    def _emit_tr(item):
                        attn_g, attnT_t, g0, gw0 = item
                        for jj in range(gw0 // P):
                            j = g0 * 4 + jj
                            pst = pstr.tile([P, P], bf16, tag="pstr", name="pst")
                            nc.tensor.transpose(
                                pst[:], attn_g[:, jj * P : (jj + 1) * P], ident_t[:]
                            )
                            nc.vector.tensor_copy(
                                attnT_t[:, j * P : (j + 1) * P], pst[:]
                            )

                    def flush_pend(all=True):
                        while pends and (all or len(pends) > pend_depth):
                            _emit_tr(pends.pop(0))

                    def emit_v_group(sg):
                        slab = load_slab(xT_d, sg * 512, 512)
                        for ss in range(4):
                            for h in range(2):
                                ps = psmm.tile([P, 512], f32, tag="psmm", name="ps")
                                for d in range(DK):
                                    nc.tensor.matmul(
                                        ps[:],
                                        slab[d][:, ss * P : (ss + 1) * P],
                                        wv[d][:, h * 512 : (h + 1) * 512],
                                        start=(d == 0),
                                        stop=(d == DK - 1),
                                    )
                                nc.vector.tensor_copy(
                                    v[sg * 4 + ss][:, h * 512 : (h + 1) * 512], ps[:]
                                )

                    def emit_scores(t):
                        nonlocal pend
                        nsb = 2 * t + 2
                        ext = nsb * P
                        G = math.ceil(ext / 512)
                        sums = statp.tile([P, G], f32, tag="sums", name="sums")
                        attnT = attnTp.tile([P, ext], bf16, tag="attnT", name="attnT")
                        for g in range(G):
                            gw = min(512, ext - g * 512)
                            ps = psmm.tile([P, gw], f32, tag="psmm", name="ps")
                            if fp8sc:
                                for jo in range(DK // 2):
                                    lhs = qT8[jo][:].rearrange(
                                        "p (two m) -> p two m", two=2
                                    )[:, :, t * P : (t + 1) * P]
                                    rhs = kT8[jo][:].rearrange(
                                        "p (two n) -> p two n", two=2
                                    )[:, :, g * 512 : g * 512 + gw]
                                    nc.tensor.matmul(
                                        ps[:],
                                        lhs,
                                        rhs,
                                        start=(jo == 0),
                                        stop=(jo == DK // 2 - 1),
                                        perf_mode=mybir.MatmulPerfMode.DoubleRow,
                                    )
                            else:
                                for o in range(DK):
                                    nc.tensor.matmul(
                                        ps[:],
                                        qT[o][:, t * P : (t + 1) * P],
                                        kT[o][:, g * 512 : g * 512 + gw],
                                        start=(o == 0),
                                        stop=(o == DK - 1),
                                    )
                            if g == G - 1:
                                nc.vector.tensor_add(
                                    ps[:, gw - 2 * P : gw],
                                    ps[:, gw - 2 * P : gw],
                                    mask_t[:],
                                )
                            attn_g = attnp.tile([P, gw], bf16, tag="attn", name="attn_g")
                            nc.scalar.activation(
                                attn_g[:],
                                ps[:],
                                mybir.ActivationFunctionType.Exp,
                                scale=(1.0 / (32.0 * SQ * SQ)) if fp8sc else (1.0 / 32.0),
                                accum_out=sums[:, g : g + 1],
                            )
                            pends.append((attn_g, attnT, g, gw))
                            flush_pend(all=False)
                        tot = statp.tile([P, 1], f32, tag="tot", name="tot")
                        nc.vector.reduce_sum(tot[:], sums[:], axis=mybir.AxisListType.X)
                        recip = statp.tile([P, 1], f32, tag="recip", name="recip")
                        nc.vector.reciprocal(recip[:], tot[:])
                        blocks[t] = (attnT, recip, nsb)

                    def emit_pv(t):
                        attnT_p, recip_p, nsb_p = blocks.pop(t)
                        outst = outp.tile([P, D], f32, tag="outst", name="outst")
                        for h in range(2):
                            ps = pspv.tile([P, 512], f32, tag="pspv", name="pspv")
                            for j in range(nsb_p):
                                nc.tensor.matmul(
                                    ps[:],
                                    attnT_p[:, j * P : (j + 1) * P],
                                    v[j][:, h * 512 : (h + 1) * 512],
                                    start=(j == 0),
                                    stop=(j == nsb_p - 1),
                                )
                            nc.vector.tensor_scalar_mul(
                                outst[:, h * 512 : (h + 1) * 512], ps[:], recip_p[:]
                            )
                        nc.sync.dma_start(out_d[t], outst[:])

                    emit_v_group(0)
                    emit_scores(0)
                    for sg in range(1, 8):
                        emit_v_group(sg)
                        emit_scores(2 * sg - 1)
                        emit_pv(2 * sg - 2)
                        emit_scores(2 * sg)
                        emit_pv(2 * sg - 1)
                    emit_scores(15)
                    emit_pv(14)
                    flush_pend()
                    emit_pv(15)
                    continue

                # ---- V projection: v[s, o] for all 4096 rows
                wv = load_w(wvT_d)
                v = [vp.tile([P, D], bf16, tag=f"v{j}", name=f"v{j}") for j in range(S // P)]
                for sg in range(8):
                    slab = load_slab(xT_d, sg * 512, 512)
                    for sh in range(8):
                        ss, h = sh // 2, sh % 2
                        ps = psmm.tile([P, 512], f32, tag="psmm", name="ps")
                        for d in range(DK):
                            nc.tensor.matmul(
                                ps[:],
                                slab[d][:, ss * P : (ss + 1) * P],
                                wv[d][:, h * 512 : (h + 1) * 512],
                                start=(d == 0),
                                stop=(d == DK - 1),
                            )
                        if variant == "v1" and ss % 2 == 1:
                            nc.scalar.copy(
                                v[sg * 4 + ss][:, h * 512 : (h + 1) * 512], ps[:]
                            )
                        else:
                            nc.vector.tensor_copy(
                                v[sg * 4 + ss][:, h * 512 : (h + 1) * 512], ps[:]
                            )

                if phase == "proj":
                    outst = outp.tile([P, D], f32, tag="outst", name="outst")
                    nc.vector.tensor_copy(outst[:, 0:512], qT[0][:, 0:512])
                    nc.vector.tensor_copy(outst[:, 512:768], kT[0][:, 0:256])
                    nc.vector.tensor_copy(outst[:, 768:1024], v[0][:, 0:256])
                    nc.sync.dma_start(out_d[0], outst[:])
                    continue

                # ---- attention, software-pipelined by one query block.
                # Transposes lag their score group by one group so the PE
                # never waits on the ACT exp of the group it just computed.
                state = None
                pend = None  # (attn_g tile, attnT tile, group idx, width)

                def flush_pend():
                    nonlocal pend
                    if pend is None:
                        return
                    attn_g, attnT_t, g0, gw0 = pend
                    for jj in range(gw0 // P):
                        j = g0 * 4 + jj
                        pst = pstr.tile([P, P], bf16, tag="pstr", name="pst")
                        nc.tensor.transpose(
                            pst[:], attn_g[:, jj * P : (jj + 1) * P], ident_t[:]
                        )
                        nc.vector.tensor_copy(
                            attnT_t[:, j * P : (j + 1) * P], pst[:]
                        )
                    pend = None

                for t in range(NQB + 1):
                    new_state = None
                    if t < NQB:
                        nsb = 2 * t + 2          # 128-col key blocks in extent
                        ext = nsb * P
                        G = math.ceil(ext / 512)  # 512-col score groups
                        sums = statp.tile([P, G], f32, tag="sums", name="sums")
                        attnT = attnTp.tile([P, ext], bf16, tag="attnT", name="attnT")
                        for g in range(G):
                            gw = min(512, ext - g * 512)
                            ps = psmm.tile([P, gw], f32, tag="psmm", name="ps")
                            for o in range(DK):
                                nc.tensor.matmul(
                                    ps[:],
                                    qT[o][:, t * P : (t + 1) * P],
                                    kT[o][:, g * 512 : g * 512 + gw],
                                    start=(o == 0),
                                    stop=(o == DK - 1),
                                )
                            if g == G - 1:
                                nc.vector.tensor_add(
                                    ps[:, gw - 2 * P : gw],
                                    ps[:, gw - 2 * P : gw],
                                    mask_t[:],
                                )
                            attn_g = attnp.tile([P, gw], bf16, tag="attn", name="attn_g")
                            nc.scalar.activation(
                                attn_g[:],
                                ps[:],
                                mybir.ActivationFunctionType.Exp,
                                scale=1.0 / 32.0,
                                accum_out=sums[:, g : g + 1],
                            )
                            pends.append((attn_g, attnT, g, gw))
                            flush_pend(all=False)
                        tot = statp.tile([P, 1], f32, tag="tot", name="tot")
                        nc.vector.reduce_sum(tot[:], sums[:], axis=mybir.AxisListType.X)
                        recip = statp.tile([P, 1], f32, tag="recip", name="recip")
                        nc.vector.reciprocal(recip[:], tot[:])
                        new_state = (attnT, recip, nsb, t)

                    if state is not None and phase == "scores":
                        if t == NQB:
                            flush_pend()
                        attnT_p, recip_p, nsb_p, tp = state
                        outst = outp.tile([P, D], f32, tag="outst", name="outst")
                        nc.vector.tensor_copy(outst[:, 0:256], attnT_p[:, 0:256])
                        nc.vector.tensor_copy(outst[:, 512:513], recip_p[:])
                        nc.sync.dma_start(out_d[tp], outst[:])
                    elif state is not None:
                        if t == NQB:
                            flush_pend()
                        attnT_p, recip_p, nsb_p, tp = state
                        outst = outp.tile([P, D], f32, tag="outst", name="outst")
                        for h in range(2):
                            ps = pspv.tile([P, 512], f32, tag="pspv", name="pspv")
                            for j in range(nsb_p):
                                nc.tensor.matmul(
                                    ps[:],
                                    attnT_p[:, j * P : (j + 1) * P],
                                    v[j][:, h * 512 : (h + 1) * 512],
                                    start=(j == 0),
                                    stop=(j == nsb_p - 1),
                                )
                            nc.vector.tensor_scalar_mul(
                                outst[:, h * 512 : (h + 1) * 512], ps[:], recip_p[:]
                            )
                        nc.sync.dma_start(out_d[tp], outst[:])

                    state = new_state

    nc.compile()
    return nc


def _get_program(reps=1, phase="full", variant=None):
    if variant is None:
        variant = VARIANT
    key = ("nc", reps, phase, variant)
    if key not in _CACHE:
        _CACHE[key] = _build_program(reps, phase, variant)
    return _CACHE[key]


def _make_in_maps(x, Wq, Wk, Wv):
    bf16 = ml_dtypes.bfloat16
    wqT = np.ascontiguousarray(Wq.T).astype(bf16).reshape(DK, P, D)
    wkT = np.ascontiguousarray(Wk.T).astype(bf16).reshape(DK, P, D)
    wvT = np.ascontiguousarray(Wv.T).astype(bf16).reshape(DK, P, D)
    ident = np.eye(P, dtype=np.float32).astype(bf16)
    masks = []
    tri = np.triu(np.full((P, P), NEG, np.float32), k=1)  # [i,j]=NEG where j>i
    for k in range(2):
        m = np.zeros((P, 2 * P), np.float32)
        if k == 0:
            m[:, :P] = tri
            m[:, P:] = NEG
        else:
            m[:, P:] = tri
        masks.append(m.astype(bf16))

    kvex = VARIANT in ("v13",)
    in_maps = []
    for c in range(NCORES):
        b, k = c // 2, c % 2
        xb_T = np.ascontiguousarray(x[b].T.astype(bf16))  # [D, S]
        q_cols = np.concatenate(
            [np.arange((2 * t + k) * P, (2 * t + k + 1) * P) for t in range(NQB)]
        )
        xTq = np.ascontiguousarray(xb_T[:, q_cols])
        m = {
            "xTq": xTq.reshape(DK, P, S // 2),
            "wqT": wqT,
            "wkT": wkT,
            "wvT": wvT,
            "mask": masks[k],
            "ident": ident,
        }
        if kvex:
            xTh = np.ascontiguousarray(xb_T[:, k * (S // 2) : (k + 1) * (S // 2)])
            m["xTh"] = xTh.reshape(DK, P, S // 2)
        else:
            m["xT"] = xb_T.reshape(DK, P, S)
        in_maps.append(m)
    return in_maps


def kernel(x, Wq, Wk, Wv):
    x = np.asarray(x, dtype=np.float32)
    Wq = np.asarray(Wq, dtype=np.float32)
    Wk = np.asarray(Wk, dtype=np.float32)
    Wv = np.asarray(Wv, dtype=np.float32)

    nc = _get_program()
    in_maps = _make_in_maps(x, Wq, Wk, Wv)
    try:
        res = run_bass_kernel_spmd(nc, in_maps, list(range(NCORES)))
    except ModuleNotFoundError:
        # profiling hook unavailable in this environment; run untraced
        import os as _os
        _os.environ["BASS_NEVER_TRACE"] = "1"
        res = run_bass_kernel_spmd(nc, in_maps, list(range(NCORES)))

    out = np.empty((B, S, D), np.float32)
    for c in range(NCORES):
        b, k = c // 2, c % 2
        oc = res.results[c]["out"]  # [NQB, P, D]
        for t in range(NQB):
            g = 2 * t + k
            out[b, g * P : (g + 1) * P, :] = oc[t]
    return out

